# revision 8
# baseline (speedup 1.0000x reference)
"""Attention-pooling kernel for TRN2 (8 NeuronCores, data-parallel over batch).

Problem (nn_AttentionPooling3): x [16, 4096, 1024] f32; per head h of 8,
logit[b,h,t] = x[b,t,h*128:(h+1)*128] @ (Q[h] @ key_p[h]) / sqrt(64);
attn = softmax over t; out[b, h*128:(h+1)*128] = sum_t attn * x-slice.

Strategy per core (2 batches/core), v2 — engine-balanced fp16 pipeline:
- The fp32 weighted-sum matmuls of v1 ran in multi-pass fp32 mode and made
  the PE the serializer (163us busy); every other engine (and the DMA,
  which sustains ~370 GB/s on a single HWDGE queue) stalled behind it.
- v2 keeps all hot tensors 2-byte: ScalarE converts each x unit to fp16
  (xh), DVE multiplies xh*wh -> prod fp16 in its 2x mode, and prod serves
  as BOTH the softmax-logit reduce input and the PE's moving tensor. The
  PE computes sum_t e * (x*w) and the host divides the tiny [B, F] output
  by w afterward -- the w factor cancels exactly, so no precision loss.
- Logits (grouped 128-wide reduces) stay fp32 and are split DVE/GP to
  balance: GP is dtype-blind at ~8.1us/unit, DVE ~4.3us/unit but also
  owns all muls (~2.1us/unit in 2x mode).
- exp on ScalarE -> e in bf16 (fp16 would overflow: logits reach +63; no
  max subtraction needed since softmax is shift-invariant and fp32/bf16
  exponent range covers e^63). PE: lhsT = e bf16, rhs = prod fp16.
- Normalizer: per-chunk N=1 matmuls vs a bf16 ones column into s_ps.
Engine budget: DMA ~93us (roofline: 33.6MB @ 360GB/s), Scalar ~63us,
DVE ~69us, GP ~65us, PE ~45us -> DMA-bound.
"""

import math

import numpy as np

import concourse.bass as bass
import concourse.mybir as mybir
import concourse.tile as tile
from concourse.bass_utils import run_bass_kernel_spmd

B, T, F = 16, 4096, 1024
H, V, KD = 8, 128, 64
NCORES = 8
BL = B // NCORES            # batches per core: 2
NCH = 4                     # 128-row chunks per unit
NCHUNKS = T // 128          # 32
FP32 = mybir.dt.float32
FP16 = mybir.dt.float16
BF16 = mybir.dt.bfloat16


# Work items per batch: (first-128-chunk, n-chunks, mul-engine).
# Free-axis grouped reduces are DVE-only on TRN2, so DVE owns all the
# reduces (2-stage, ~2.4us/unit) plus half the muls; GP (dtype-blind,
# ~8.1us/unit) takes the other half of the muls. First item of batch 0 is
# a single chunk so the pipeline primes after a 0.5MB DMA; the tail of
# batch 1 is fine-grained so the last unit's chain is short.
def _items_for(b):
    if b == 0:
        return [
            (0, 1, "ve"), (1, 3, "ve"),
            (4, 4, "gp"), (8, 4, "gp"), (12, 4, "gp"), (16, 4, "gp"),
            (20, 4, "ve"), (24, 4, "ve"), (28, 4, "ve"),
        ]
    return [
        (0, 4, "gp"), (4, 4, "gp"), (8, 4, "gp"), (12, 4, "gp"),
        (16, 4, "ve"), (20, 4, "ve"), (24, 4, "ve"),
        (28, 2, "ve"), (30, 1, "ve"), (31, 1, "ve"),
    ]


def _build_nc():
    nc = bass.Bass()
    x_d = nc.declare_dram_parameter("x", [BL, T, F], FP32, isOutput=False)
    wh_d = nc.declare_dram_parameter("wh", [128, F], FP16, isOutput=False)
    y_d = nc.declare_dram_parameter("y", [BL, H, F], FP32, isOutput=True)

    with tile.TileContext(nc) as tc:
        with (
            tc.tile_pool(name="const", bufs=1) as const_pool,
            tc.tile_pool(name="xin", bufs=3) as xpool,
            tc.tile_pool(name="xh", bufs=3) as xhpool,
            tc.tile_pool(name="prod", bufs=4) as ppool,
            tc.tile_pool(name="small", bufs=4) as small,
            tc.tile_pool(name="yout", bufs=2) as ypool,
            tc.tile_pool(name="acc", bufs=2, space="PSUM") as psum_pool,
        ):
            # wh loads once (256KB) on the Scalar HWDGE queue so it doesn't
            # delay unit 0's x load on the Sync queue.
            wh_sb = const_pool.tile([128, F], FP16)
            nc.scalar.dma_start(out=wh_sb, in_=wh_d[:, :])
            ones_sb = const_pool.tile([128, 1], BF16)
            nc.vector.memset(ones_sb, 1.0)

            for b in range(BL):
                pooled_ps = psum_pool.tile([H, F], FP32)
                s_ps = psum_pool.tile([H, 1], FP32)
                items = _items_for(b)
                for ch0, nch, eng in items:
                    xt = xpool.tile([128, NCH, F], FP32, name="xt")
                    xt_v = xt[:, :nch, :]
                    # All x loads on the Sync HWDGE queue; a single queue's
                    # descriptors spread over all 16 DMA engines (~370GB/s).
                    nc.sync.dma_start(
                        out=xt_v,
                        in_=x_d[
                            b, ch0 * 128 : (ch0 + nch) * 128, :
                        ].rearrange("(n p) f -> p n f", p=128),
                    )
                    xh = xhpool.tile([128, NCH, F], FP16, name="xh")
                    xh_v = xh[:, :nch, :]
                    nc.scalar.activation(
                        out=xh_v,
                        in_=xt_v,
                        func=mybir.ActivationFunctionType.Copy,
                    )
                    prod = ppool.tile([128, NCH, F], FP16, name="prod")
                    prod_v = prod[:, :nch, :]
                    wh_bc = bass.AP(
                        tensor=wh_sb.tensor,
                        offset=wh_sb.offset,
                        ap=[wh_sb.ap[0], [0, nch], wh_sb.ap[1]],
                    )
                    # fp16 in / fp16 out, packed: DVE runs this in 2x mode;
                    # on GP the rate is dtype-independent.
                    mul_eng = nc.vector if eng == "ve" else nc.gpsimd
                    mul_eng.tensor_mul(prod_v, xh_v, wh_bc)
                    # Two-stage grouped reduce on DVE: stage A sums groups of
                    # 16 in fp16 (all-2-byte packed -> 2x mode; partial sums
                    # add <=2e-3 abs to logits, validated off-line vs the
                    # 2e-2 gate), stage B finishes in fp32.
                    part_u = small.tile([128, NCH, H * 8], FP16, name="part_u")
                    with nc.allow_low_precision(
                        reason="fp16 16-wide partial sums; adds <=2e-3 to logits"
                    ):
                        nc.vector.tensor_reduce(
                            part_u[:, :nch, :],
                            prod_v.rearrange(
                                "p n (g v2) -> p n g v2", v2=16
                            ),
                            axis=mybir.AxisListType.X,
                            op=mybir.AluOpType.add,
                        )
                    logits_u = small.tile([128, NCH, H], FP32, name="logits_u")
                    nc.vector.tensor_reduce(
                        logits_u[:, :nch, :],
                        part_u[:, :nch, :].rearrange(
                            "p n (h g) -> p n h g", g=8
                        ),
                        axis=mybir.AxisListType.X,
                        op=mybir.AluOpType.add,
                    )
                    e_u = small.tile([128, NCH, H], BF16, name="e_u")
                    nc.scalar.activation(
                        out=e_u[:, :nch, :],
                        in_=logits_u[:, :nch, :],
                        func=mybir.ActivationFunctionType.Exp,
                    )
                    # Group matmuls by PSUM bank (all low halves, then all
                    # high halves): per-MM bank alternation causes HAM
                    # re-throttle and blocks MM pipelining.
                    for half in range(2):
                        lo, hi = half * 512, half * 512 + 512
                        for n in range(nch):
                            ch = ch0 + n
                            nc.tensor.matmul(
                                pooled_ps[:, lo:hi],
                                e_u[:, n, :],
                                prod[:, n, lo:hi],
                                start=ch == 0,
                                stop=ch == NCHUNKS - 1,
                            )
                    for n in range(nch):
                        ch = ch0 + n
                        nc.tensor.matmul(
                            s_ps,
                            e_u[:, n, :],
                            ones_sb,
                            start=ch == 0,
                            stop=ch == NCHUNKS - 1,
                        )
                r_sb = small.tile([H, 1], FP32)
                nc.vector.reciprocal(r_sb, s_ps)
                y_sb = ypool.tile([H, F], FP32)
                nc.scalar.activation(
                    out=y_sb,
                    in_=pooled_ps,
                    func=mybir.ActivationFunctionType.Copy,
                    scale=r_sb,
                )
                nc.sync.dma_start(out=y_d[b], in_=y_sb)
    return nc


def _split_multiwaits(nc, limit=1):
    """This container's walrus accepts at most `limit` sync-wait commands per
    instruction ("Too many sync wait commands" otherwise). Tile attaches up to
    ~12. Move excess waits onto preceding same-engine NoOps — semantics are
    unchanged (waits are AND conditions that block the engine either way)."""
    for fn in nc.m.functions:
        for blk in fn.blocks:
            new = []
            for inst in blk.instructions:
                si = getattr(inst, "sync_info", None)
                ow = list(si.on_wait) if si is not None and si.on_wait else []
                if len(ow) > limit:
                    extra, keep = ow[:-limit], ow[-limit:]
                    for i in range(0, len(extra), limit):
                        new.append(
                            mybir.InstNoOp(
                                name=f"{inst.name}-wsplit{i}",
                                engine=inst.engine,
                                ins=[],
                                outs=[],
                                sync_info=mybir.SyncInfo(
                                    on_wait=extra[i : i + limit], on_update=[]
                                ),
                            )
                        )
                    inst.sync_info = mybir.SyncInfo(
                        on_wait=keep, on_update=si.on_update
                    )
                new.append(inst)
            blk.instructions = new


_NC = None


def _get_nc():
    global _NC
    if _NC is None:
        _NC = _build_nc()
        _split_multiwaits(_NC)
    return _NC


def _fold_weights(Q, key_p):
    w = np.einsum(
        "hvk,hk->hv", np.asarray(Q, np.float32), np.asarray(key_p, np.float32)[:, :, 0]
    ) / np.float32(math.sqrt(KD))
    return w.reshape(H * V).astype(np.float32)


def _run(x, Q, key_p, trace=False, tmpdir=None):
    x = np.ascontiguousarray(np.asarray(x, np.float32))
    w_flat = _fold_weights(Q, key_p)
    wh = np.tile(w_flat.astype(np.float16).reshape(1, H * V), (128, 1))
    nc = _get_nc()
    in_maps = [
        {"x": x[c * BL : (c + 1) * BL], "wh": wh} for c in range(NCORES)
    ]
    res = run_bass_kernel_spmd(
        nc, in_maps, list(range(NCORES)), trace=trace, tmpdir=tmpdir
    )
    # Kernel returns sum_t e*(x*w) / sum_t e; the host divides out the fp32
    # w (the fp16 rounding of w cancels exactly between logits and pooled).
    y = np.empty((B, F), np.float32)
    wh32 = wh[0].astype(np.float32)  # the exact fp16 values used on-chip
    for c in range(NCORES):
        yc = res.results[c]["y"]  # [BL, H, F]
        for b in range(BL):
            for h in range(H):
                sl = slice(h * V, (h + 1) * V)
                y[c * BL + b, sl] = yc[b, h, sl] / wh32[sl]
    return y, res


def kernel(**inputs):
    y, _ = _run(inputs["x"], inputs["Q"], inputs["key_p"])
    return y
